# revision 2
# baseline (speedup 1.0000x reference)
"""2-layer GCN on 8 TRN2 NeuronCores via Bass/Tile.

Strategy (per spec sharding_hint): dst-shard nodes across 8 cores; edges
partitioned by destination; small weight matrices replicated. Three SPMD
launches with host-side shard exchange between them:
  A: supT = (x_shard @ W1)^T in bf16      (host transposes to the table)
  B: h = relu(agg1(sup1)+b1); sup2 = h @ W2   (dst-sharded edges,
     dma_gather per edge from replicated bf16 sup1 table)
  C: out = agg2(sup2) + b2

Aggregation: per 128-edge block, gather rows tab[src] (SWDGE dma_gather,
4 queues, groups of GW=8 dst windows per call), build selection matrix
S[e,d] = w_e * (dstlocal_e == d) in ONE fused DVE tensor_scalar
(out = (iota == dloc) * ew, per-partition scalars), TensorE bf16 matmul
psum[d,f] += S^T @ msgs accumulated over a 128-dst window.
"""
import sys

sys.path.insert(0, "/opt/trn_rl_repo")
import numpy as np
import ml_dtypes
import concourse.bacc as bacc
import concourse.bass as bass
import concourse.mybir as mybir
import concourse.tile as tile
from concourse.bass_utils import run_bass_kernel_spmd
from concourse.library_config import mlp

dt = mybir.dt
F32 = dt.float32
BF16 = dt.bfloat16
BF_NP = ml_dtypes.bfloat16
NCORES = 8
P = 128
GROUP_W = 8  # dst windows per gather-call group


# ---------------------------------------------------------------- host prep
def bucket_edges(src, dst, ew, n_nodes, n_chunks, chunk_rows, shard, nwin):
    """Per-core edge buckets (window = 128 local dsts, chunk = src range).

    Returns per-core arrays (idx wrapped int16, dstloc f32, weight f32,
    all [*, 128]-blocked) plus the uniform block-count table B[w][c].
    Padding uses idx 0 / weight 0 (gathers row 0, killed by S=0), so
    grouped gather calls see a fully valid index stream.
    """
    counts = np.zeros((NCORES, nwin, n_chunks), dtype=np.int64)
    core = dst // shard
    dloc = dst - core * shard
    win = dloc // P
    chunk = src // chunk_rows
    for k in range(NCORES):
        m = core == k
        np.add.at(counts[k], (win[m], chunk[m]), 1)
    B = np.maximum.reduce([np.ceil(counts[k] / P).astype(np.int64) for k in range(NCORES)])
    nblk_tot = int(B.sum())
    nidx_tot = nblk_tot * P

    per_core = []
    order = np.lexsort((chunk, win, core))
    srt = {"src": src[order], "ew": ew[order], "dloc": dloc[order],
           "core": core[order], "win": win[order], "chunk": chunk[order]}
    for k in range(NCORES):
        idx_arr = np.zeros(nidx_tot, dtype=np.int16)
        dloc_arr = np.zeros(nidx_tot, dtype=np.float32)
        w_arr = np.zeros(nidx_tot, dtype=np.float32)
        sel = srt["core"] == k
        s_src, s_ew, s_dloc = srt["src"][sel], srt["ew"][sel], srt["dloc"][sel]
        s_win, s_chunk = srt["win"][sel], srt["chunk"][sel]
        # bucket start pointers into this core's (win, chunk)-sorted edges
        bstart = np.zeros((nwin, n_chunks + 1), dtype=np.int64)
        csum = 0
        for w in range(nwin):
            for c in range(n_chunks):
                bstart[w, c] = csum
                csum += counts[k, w, c]
            bstart[w, n_chunks] = csum
        # group-major layout: for g (GW windows): for c: for w in g
        GW = GROUP_W
        ngrp = (nwin + GW - 1) // GW
        pos = 0
        for g in range(ngrp):
            for c in range(n_chunks):
                for w in range(g * GW, min((g + 1) * GW, nwin)):
                    n = counts[k, w, c]
                    cap = B[w, c] * P
                    e0 = bstart[w, c]
                    e1 = e0 + n
                    idx_arr[pos:pos + n] = (s_src[e0:e1] - c * chunk_rows).astype(np.int16)
                    dloc_arr[pos:pos + n] = (s_dloc[e0:e1] - w * P).astype(np.float32)
                    w_arr[pos:pos + n] = s_ew[e0:e1]
                    # padding: idx 0, weight 0, dstloc 0 (weight 0 kills it)
                    pos += cap
        idx_wrapped = np.tile(idx_arr.reshape(-1, 16).T, (8, 1)).copy()  # [128, n/16]
        per_core.append({
            "idx": idx_wrapped,
            "dloc": dloc_arr.reshape(-1, P).T.copy(),  # [128, nblk]
            "w": w_arr.reshape(-1, P).T.copy(),        # [128, nblk]
        })
    return per_core, B


# ---------------------------------------------------------------- phase A
def build_phase_a(shard, nfeat, nhid):
    """supT shard = (x_shard @ W1)^T in bf16; host transposes to table."""
    nc = bacc.Bacc("TRN2")
    xT = nc.declare_dram_parameter("xT", [nfeat, shard], BF16, isOutput=False)
    W1 = nc.declare_dram_parameter("W1", [nfeat, nhid], BF16, isOutput=False)
    supT = nc.declare_dram_parameter("supT", [nhid, shard], BF16, isOutput=True)
    kt = nfeat // P
    NT = 512
    ntiles = (shard + NT - 1) // NT
    with tile.TileContext(nc) as tc:
        with (
            tc.tile_pool(name="const", bufs=1) as cpool,
            tc.tile_pool(name="work", bufs=3) as wpool,
            tc.tile_pool(name="psum", bufs=2, space="PSUM") as ppool,
        ):
            w1_sb = [cpool.tile([P, nhid], BF16, tag=f"w1_{k}", name=f"w1_{k}") for k in range(kt)]
            for k in range(kt):
                nc.sync.dma_start(w1_sb[k][:], W1[k * P:(k + 1) * P, :])
            for t in range(ntiles):
                n0 = t * NT
                n = min(NT, shard - n0)
                xt_sb = [wpool.tile([P, NT], BF16, tag=f"xt_{k}", name=f"xt_{k}_{t}") for k in range(kt)]
                for k in range(kt):
                    nc.sync.dma_start(xt_sb[k][:, :n], xT[k * P:(k + 1) * P, n0:n0 + n])
                psT = ppool.tile([P, NT], F32, tag="ps")
                for k in range(kt):
                    nc.tensor.matmul(psT[:, :n], lhsT=w1_sb[k][:],
                                     rhs=xt_sb[k][:, :n],
                                     start=(k == 0), stop=(k == kt - 1))
                supT_sb = wpool.tile([P, NT], BF16, tag="supT")
                nc.vector.tensor_copy(out=supT_sb[:, :n], in_=psT[:, :n])
                nc.sync.dma_start(supT[:, n0:n0 + n], supT_sb[:, :n])
    nc.compile()
    return nc


# ---------------------------------------------------------------- phases B/C
def build_agg(shard, n_chunks, chunk_rows, B, n_nodes, second, nhid):
    """Aggregation kernel over a bf16 [n_nodes, 128] gather table.

    second=False (phase B): epilogue h=relu(agg+b1); sup2 = h @ W2pad
      -> out [shard, 64] bf16.
    second=True (phase C): agg over first 64 cols; out = agg + b2
      -> out [shard, 64] f32.
    """
    nwin = B.shape[0]
    nblk = int(B.sum())
    nidx = nblk * P
    nc = bacc.Bacc("TRN2", num_swdge_queues=4)
    tabw = 128
    mmw = 64 if second else 128
    aggw = 64 if second else 128
    tab = nc.declare_dram_parameter("tab", [n_nodes, tabw], BF16, isOutput=False)
    idxs = nc.declare_dram_parameter("idxs", [P, nidx // 16], dt.int16, isOutput=False)
    dloc = nc.declare_dram_parameter("dloc", [P, nblk], F32, isOutput=False)
    ew = nc.declare_dram_parameter("ew", [P, nblk], F32, isOutput=False)
    iota = nc.declare_dram_parameter("iota", [P, P], BF16, isOutput=False)
    brep = nc.declare_dram_parameter("brep", [P, aggw], F32, isOutput=False)
    if not second:
        ident = nc.declare_dram_parameter("ident", [P, P], F32, isOutput=False)
        W2 = nc.declare_dram_parameter("W2", [nhid, 64], BF16, isOutput=False)
    out = nc.declare_dram_parameter("out", [shard, 64], F32 if second else BF16,
                                    isOutput=True)

    # grouped-call layout: for g (GW windows): for c: for w in g: B[w,c] blocks
    GW = GROUP_W
    ngrp = (nwin + GW - 1) // GW
    call_off = np.zeros((ngrp, n_chunks), dtype=np.int64)   # call start block
    call_nb = np.zeros((ngrp, n_chunks), dtype=np.int64)    # blocks per call
    woff = np.zeros((nwin, n_chunks), dtype=np.int64)       # window offset within call
    acc = 0
    for g in range(ngrp):
        ws = range(g * GW, min((g + 1) * GW, nwin))
        for c in range(n_chunks):
            call_off[g, c] = acc
            o = 0
            for w in ws:
                woff[w, c] = o
                o += B[w, c]
            call_nb[g, c] = o
            acc += o
    Bg_max = int(call_nb.max())
    Bwc_max = int(B.max())

    # message-tile rotation depth: fit SBUF (~24 MB budget)
    fixed = nidx // 16 * 2 * P + nblk * 4 * P * 2 + 6 * P * P * 4
    NS = 2
    while NS < 4 and fixed + n_chunks * (NS + 1) * P * Bg_max * tabw * 2 < 21 * 2**20:
        NS += 1

    with tile.TileContext(nc) as tc:
        nc.gpsimd.load_library(mlp)
        with (
            tc.tile_pool(name="const", bufs=1) as cpool,
            tc.tile_pool(name="s", bufs=4) as spool,
            tc.tile_pool(name="epi", bufs=3) as epool,
            tc.tile_pool(name="psum", bufs=3, space="PSUM") as ppool,
            tc.tile_pool(name="psum2", bufs=2, space="PSUM") as p2pool,
        ):
            idx_sb = cpool.tile([P, nidx // 16], dt.int16)
            nc.sync.dma_start(idx_sb[:], idxs[:])
            dloc_sb = cpool.tile([P, nblk], F32)
            nc.sync.dma_start(dloc_sb[:], dloc[:])
            ew_sb = cpool.tile([P, nblk], F32)
            nc.sync.dma_start(ew_sb[:], ew[:])
            iota_sb = cpool.tile([P, P], BF16)
            nc.sync.dma_start(iota_sb[:], iota[:])
            brep_sb = cpool.tile([P, aggw], F32)
            nc.sync.dma_start(brep_sb[:], brep[:])
            if not second:
                id_sb = cpool.tile([P, P], F32)
                nc.sync.dma_start(id_sb[:], ident[:])
                w2_sb = cpool.tile([nhid, 64], BF16)
                nc.sync.dma_start(w2_sb[:], W2[:])

            msgs_tiles = [[cpool.tile([P, Bg_max, tabw], BF16, tag=f"mt_{c}_{s}", name=f"mt_{c}_{s}")
                           for s in range(NS)] for c in range(n_chunks)]
            qn = 0
            for g in range(ngrp):
                ws = list(range(g * GW, min((g + 1) * GW, nwin)))
                gm = {}
                for c in range(n_chunks):
                    nbc = int(call_nb[g, c])
                    if nbc == 0:
                        continue
                    off = int(call_off[g, c])
                    msgs = msgs_tiles[c][g % NS]
                    gm[c] = (msgs, off)
                    nc.gpsimd.dma_gather(
                        msgs[:, :nbc, :], tab[c * chunk_rows:min((c + 1) * chunk_rows, n_nodes), :],
                        idx_sb[:, off * 8:(off + nbc) * 8],
                        nbc * P, nbc * P, tabw, single_packet=False, queue_num=qn)
                    qn = (qn + 1) % 4
                for w in ws:
                    psw = ppool.tile([P, mmw], F32, tag="psw", name=f"psw_{w}")
                    first = True
                    nb_w = int(B[w].sum())
                    if nb_w == 0:
                        nc.vector.memset(psw[:], 0.0)
                    done = 0
                    for c in range(n_chunks):
                        nb = int(B[w, c])
                        if nb == 0:
                            continue
                        msgs, off = gm[c]
                        wo = int(woff[w, c])
                        blk0 = off + wo
                        S4 = spool.tile([P, Bwc_max, P], BF16, tag="s", name=f"s_{w}_{c}")
                        for b in range(nb):
                            nc.vector.tensor_scalar(
                                out=S4[:, b, :], in0=iota_sb[:],
                                scalar1=dloc_sb[:, blk0 + b:blk0 + b + 1],
                                scalar2=ew_sb[:, blk0 + b:blk0 + b + 1],
                                op0=mybir.AluOpType.is_equal,
                                op1=mybir.AluOpType.mult)
                            done += 1
                            nc.tensor.matmul(psw[:, :], lhsT=S4[:, b, :], rhs=msgs[:, wo + b, :mmw],
                                             start=first, stop=(done == nb_w))
                            first = False
                    rows = min(P, shard - w * P)
                    if second:
                        o_sb = epool.tile([P, 64], F32, tag="o", name=f"o_{w}")
                        nc.vector.tensor_tensor(out=o_sb[:], in0=psw[:], in1=brep_sb[:], op=mybir.AluOpType.add)
                        nc.sync.dma_start(out[w * P:w * P + rows, :], o_sb[:rows, :])
                    else:
                        hb = epool.tile([P, aggw], F32, tag="hb", name=f"hb_{w}")
                        nc.vector.tensor_tensor(out=hb[:], in0=psw[:], in1=brep_sb[:], op=mybir.AluOpType.add)
                        h = epool.tile([P, aggw], F32, tag="h", name=f"h_{w}")
                        nc.scalar.activation(out=h[:], in_=hb[:], func=mybir.ActivationFunctionType.Relu)
                        pst = p2pool.tile([P, P], F32, tag="pst", name=f"pst_{w}")
                        nc.tensor.transpose(out=pst[:], in_=h[:], identity=id_sb[:])
                        hT = epool.tile([P, P], BF16, tag="hT", name=f"hT_{w}")
                        nc.scalar.activation(out=hT[:], in_=pst[:], func=mybir.ActivationFunctionType.Copy)
                        ps2 = p2pool.tile([P, 64], F32, tag="ps2", name=f"ps2_{w}")
                        nc.tensor.matmul(ps2[:], lhsT=hT[:], rhs=w2_sb[:], start=True, stop=True)
                        o_sb = epool.tile([P, 64], BF16, tag="o", name=f"o_{w}")
                        nc.vector.tensor_copy(out=o_sb[:], in_=ps2[:])
                        nc.sync.dma_start(out[w * P:w * P + rows, :], o_sb[:rows, :])
    nc.compile()
    return nc


# ---------------------------------------------------------------- driver
def gcn_forward(x, edge_index, edge_weight, W1, b1, W2, b2, runner=None):
    """Full forward. runner(nc, in_maps, tag) -> list of per-core output dicts."""
    if runner is None:
        def runner(nc, in_maps, tag):
            res = run_bass_kernel_spmd(nc, in_maps, core_ids=list(range(NCORES)))
            return res.results
    n_nodes, nfeat = x.shape
    nhid = W1.shape[1]
    nclass = W2.shape[1]
    shard = n_nodes // NCORES
    nwin = (shard + P - 1) // P
    chunk_rows = 25000
    n_chunks = (n_nodes + chunk_rows - 1) // chunk_rows
    src = np.asarray(edge_index[0], dtype=np.int64)
    dst = np.asarray(edge_index[1], dtype=np.int64)
    ew = np.asarray(edge_weight, dtype=np.float32)

    per_core, B = bucket_edges(src, dst, ew, n_nodes, n_chunks, chunk_rows, shard, nwin)

    iota = np.tile(np.arange(P, dtype=np.float32), (P, 1)).astype(BF_NP)
    ident = np.eye(P, dtype=np.float32)
    xT = np.ascontiguousarray(np.asarray(x, dtype=np.float32).T.astype(BF_NP))
    W1b = np.asarray(W1, np.float32).astype(BF_NP)

    # phase A
    nc_a = build_phase_a(shard, nfeat, nhid)
    ins_a = [{"xT": np.ascontiguousarray(xT[:, k * shard:(k + 1) * shard]),
              "W1": W1b} for k in range(NCORES)]
    res_a = runner(nc_a, ins_a, "A")
    sup1 = np.ascontiguousarray(
        np.concatenate([r["supT"] for r in res_a], axis=1).T)  # [n_nodes, nhid] bf16

    # phase B
    b1rep = np.tile(np.asarray(b1, np.float32)[None, :], (P, 1))
    W2pad = np.zeros((nhid, 64), np.float32)
    W2pad[:, :nclass] = np.asarray(W2, np.float32)
    W2pad = W2pad.astype(BF_NP)
    nc_b = build_agg(shard, n_chunks, chunk_rows, B, n_nodes, False, nhid)
    ins_b = [{"tab": sup1, "idxs": pc["idx"], "dloc": pc["dloc"], "ew": pc["w"],
              "iota": iota, "ident": ident, "brep": b1rep, "W2": W2pad}
             for pc in per_core]
    res_b = runner(nc_b, ins_b, "B")
    sup2 = np.concatenate([r["out"] for r in res_b], axis=0)  # [n_nodes, 64] bf16
    sup2p = np.zeros((n_nodes, 128), BF_NP)
    sup2p[:, :64] = sup2

    # phase C
    b2rep = np.zeros((P, 64), np.float32)
    b2rep[:, :nclass] = np.asarray(b2, np.float32)[None, :]
    nc_c = build_agg(shard, n_chunks, chunk_rows, B, n_nodes, True, nhid)
    ins_c = [{"tab": sup2p, "idxs": pc["idx"], "dloc": pc["dloc"], "ew": pc["w"],
              "iota": iota, "brep": b2rep}
             for pc in per_core]
    res_c = runner(nc_c, ins_c, "C")
    out = np.concatenate([r["out"] for r in res_c], axis=0)[:, :nclass]
    return np.ascontiguousarray(out.astype(np.float32))


def kernel(x, edge_index, edge_weight, W1, b1, W2, b2):
    """Harness entrypoint: FULL inputs -> FULL output [n_nodes, nclass]."""
    out = gcn_forward(np.asarray(x), np.asarray(edge_index), np.asarray(edge_weight),
                      np.asarray(W1), np.asarray(b1), np.asarray(W2), np.asarray(b2))
    return out.astype(np.float32)


# revision 8
# speedup vs baseline: 1.9994x; 1.9994x over previous
"""2-layer GCN on 8 TRN2 NeuronCores via Bass/Tile.

Strategy (per spec sharding_hint): dst-shard nodes across 8 cores; edges
partitioned by destination; small weight matrices replicated. Three SPMD
launches with host-side shard exchange between them:
  A: supT = (x_shard @ W1)^T in bf16      (host transposes to the table)
  B: h = relu(agg1(sup1)+b1); sup2 = h @ W2   (dst-sharded edges,
     dma_gather per edge from replicated bf16 sup1 table)
  C: out = agg2(sup2) + b2

Aggregation: per 128-edge block, gather rows tab[src] (SWDGE dma_gather,
4 queues, groups of GW=8 dst windows per call), build selection matrix
S[e,d] = w_e * (dstlocal_e == d) in ONE fused DVE tensor_scalar
(out = (iota == dloc) * ew, per-partition scalars), TensorE bf16 matmul
psum[d,f] += S^T @ msgs accumulated over a 128-dst window.
"""
import sys

sys.path.insert(0, "/opt/trn_rl_repo")
import numpy as np
import ml_dtypes
import concourse.bacc as bacc
import concourse.bass as bass
import concourse.mybir as mybir
import concourse.tile as tile
from concourse.bass_utils import run_bass_kernel_spmd
from concourse.library_config import mlp

dt = mybir.dt
F32 = dt.float32
BF16 = dt.bfloat16
BF_NP = ml_dtypes.bfloat16
NCORES = 8
P = 128
GROUP_W = 6  # dst windows per gather-call group


# ---------------------------------------------------------------- host prep
def bucket_edges(src, dst, ew, n_nodes, n_chunks, chunk_rows, shard, nwin):
    """Per-core edge buckets (window = 128 local dsts, chunk = src range).

    Returns per-core arrays (idx wrapped int16, dstloc f32, weight f32,
    all [*, 128]-blocked) plus the uniform block-count table B[w][c].
    Padding uses idx 0 / weight 0 (gathers row 0, killed by S=0), so
    grouped gather calls see a fully valid index stream.
    """
    counts = np.zeros((NCORES, nwin, n_chunks), dtype=np.int64)
    core = dst // shard
    dloc = dst - core * shard
    win = dloc // P
    chunk = src // chunk_rows
    for k in range(NCORES):
        m = core == k
        np.add.at(counts[k], (win[m], chunk[m]), 1)
    B = np.maximum.reduce([np.ceil(counts[k] / P).astype(np.int64) for k in range(NCORES)])
    nblk_tot = int(B.sum())
    nidx_tot = nblk_tot * P

    per_core = []
    order = np.lexsort((chunk, win, core))
    srt = {"src": src[order], "ew": ew[order], "dloc": dloc[order],
           "core": core[order], "win": win[order], "chunk": chunk[order]}
    for k in range(NCORES):
        idx_arr = np.zeros(nidx_tot, dtype=np.int16)
        dloc_arr = np.zeros(nidx_tot, dtype=np.float32)
        w_arr = np.zeros(nidx_tot, dtype=np.float32)
        sel = srt["core"] == k
        s_src, s_ew, s_dloc = srt["src"][sel], srt["ew"][sel], srt["dloc"][sel]
        s_win, s_chunk = srt["win"][sel], srt["chunk"][sel]
        # bucket start pointers into this core's (win, chunk)-sorted edges
        bstart = np.zeros((nwin, n_chunks + 1), dtype=np.int64)
        csum = 0
        for w in range(nwin):
            for c in range(n_chunks):
                bstart[w, c] = csum
                csum += counts[k, w, c]
            bstart[w, n_chunks] = csum
        # group-major layout: for g (GW windows): for c: for w in g
        GW = GROUP_W
        ngrp = (nwin + GW - 1) // GW
        pos = 0
        for g in range(ngrp):
            for c in range(n_chunks):
                for w in range(g * GW, min((g + 1) * GW, nwin)):
                    n = counts[k, w, c]
                    cap = B[w, c] * P
                    e0 = bstart[w, c]
                    e1 = e0 + n
                    idx_arr[pos:pos + n] = (s_src[e0:e1] - c * chunk_rows).astype(np.int16)
                    dloc_arr[pos:pos + n] = (s_dloc[e0:e1] - w * P).astype(np.float32)
                    w_arr[pos:pos + n] = s_ew[e0:e1]
                    # padding: idx 0, weight 0, dstloc 0 (weight 0 kills it)
                    pos += cap
        idx_wrapped = np.tile(idx_arr.reshape(-1, 16).T, (8, 1)).copy()  # [128, n/16]
        per_core.append({
            "idx": idx_wrapped,
            "dloc": dloc_arr.reshape(-1, P).T.astype(BF_NP),  # [128, nblk]
            "w": w_arr.reshape(-1, P).T.astype(BF_NP),        # [128, nblk]
        })
    return per_core, B


# ---------------------------------------------------------------- phase A
def build_phase_a(shard, nfeat, nhid):
    """supT shard = (x_shard @ W1)^T in bf16; host transposes to table."""
    nc = bacc.Bacc("TRN2")
    xT = nc.declare_dram_parameter("xT", [nfeat, shard], BF16, isOutput=False)
    W1 = nc.declare_dram_parameter("W1", [nfeat, nhid], BF16, isOutput=False)
    supT = nc.declare_dram_parameter("supT", [nhid, shard], BF16, isOutput=True)
    kt = nfeat // P
    NT = 512
    ntiles = (shard + NT - 1) // NT
    with tile.TileContext(nc) as tc:
        with (
            tc.tile_pool(name="const", bufs=1) as cpool,
            tc.tile_pool(name="work", bufs=3) as wpool,
            tc.tile_pool(name="psum", bufs=2, space="PSUM") as ppool,
        ):
            w1_sb = [cpool.tile([P, nhid], BF16, tag=f"w1_{k}", name=f"w1_{k}") for k in range(kt)]
            for k in range(kt):
                nc.sync.dma_start(w1_sb[k][:], W1[k * P:(k + 1) * P, :])
            for t in range(ntiles):
                n0 = t * NT
                n = min(NT, shard - n0)
                xt_sb = [wpool.tile([P, NT], BF16, tag=f"xt_{k}", name=f"xt_{k}_{t}") for k in range(kt)]
                for k in range(kt):
                    nc.sync.dma_start(xt_sb[k][:, :n], xT[k * P:(k + 1) * P, n0:n0 + n])
                psT = ppool.tile([P, NT], F32, tag="ps")
                for k in range(kt):
                    nc.tensor.matmul(psT[:, :n], lhsT=w1_sb[k][:],
                                     rhs=xt_sb[k][:, :n],
                                     start=(k == 0), stop=(k == kt - 1))
                supT_sb = wpool.tile([P, NT], BF16, tag="supT")
                nc.vector.tensor_copy(out=supT_sb[:, :n], in_=psT[:, :n])
                nc.sync.dma_start(supT[:, n0:n0 + n], supT_sb[:, :n])
    nc.compile()
    return nc


# ---------------------------------------------------------------- phases B/C
def build_agg(shard, n_chunks, chunk_rows, B, n_nodes, second, nhid):
    """Aggregation kernel over a bf16 [n_nodes, 128] gather table.

    second=False (phase B): epilogue h=relu(agg+b1); sup2 = h @ W2pad
      -> out [shard, 64] bf16.
    second=True (phase C): agg over first 64 cols; out = agg + b2
      -> out [shard, 64] f32.
    """
    nwin = B.shape[0]
    nblk = int(B.sum())
    nidx = nblk * P
    nc = bacc.Bacc("TRN2", num_swdge_queues=4)
    tabw = 128
    mmw = 64 if second else 128
    aggw = 64 if second else 128
    tab = nc.declare_dram_parameter("tab", [n_nodes, tabw], BF16, isOutput=False)
    idxs = nc.declare_dram_parameter("idxs", [P, nidx // 16], dt.int16, isOutput=False)
    dloc = nc.declare_dram_parameter("dloc", [P, nblk], BF16, isOutput=False)
    ew = nc.declare_dram_parameter("ew", [P, nblk], BF16, isOutput=False)
    iota = nc.declare_dram_parameter("iota", [P, P], BF16, isOutput=False)
    brep = nc.declare_dram_parameter("brep", [P, aggw], F32, isOutput=False)
    if not second:
        ident = nc.declare_dram_parameter("ident", [P, P], F32, isOutput=False)
        W2 = nc.declare_dram_parameter("W2", [nhid, 64], BF16, isOutput=False)
    out = nc.declare_dram_parameter("out", [shard, 64], F32 if second else BF16,
                                    isOutput=True)

    # grouped-call layout: for g (GW windows): for c: for w in g: B[w,c] blocks
    GW = GROUP_W
    ngrp = (nwin + GW - 1) // GW
    call_off = np.zeros((ngrp, n_chunks), dtype=np.int64)   # call start block
    call_nb = np.zeros((ngrp, n_chunks), dtype=np.int64)    # blocks per call
    woff = np.zeros((nwin, n_chunks), dtype=np.int64)       # window offset within call
    acc = 0
    for g in range(ngrp):
        ws = range(g * GW, min((g + 1) * GW, nwin))
        for c in range(n_chunks):
            call_off[g, c] = acc
            o = 0
            for w in ws:
                woff[w, c] = o
                o += B[w, c]
            call_nb[g, c] = o
            acc += o
    Bg_max = int(call_nb.max())
    Bwc_max = int(B.max())

    NS = 2

    with tile.TileContext(nc) as tc:
        nc.gpsimd.load_library(mlp)
        with (
            tc.tile_pool(name="const", bufs=1) as cpool,
            tc.tile_pool(name="s0", bufs=2) as s0pool,
            tc.tile_pool(name="s", bufs=2 * n_chunks) as spool,
            tc.tile_pool(name="epi", bufs=3) as epool,
            tc.tile_pool(name="psum", bufs=3, space="PSUM") as ppool,
            tc.tile_pool(name="psum2", bufs=2, space="PSUM") as p2pool,
        ):
            idx_sb = cpool.tile([P, nidx // 16], dt.int16)
            nc.sync.dma_start(idx_sb[:], idxs[:])
            dloc_sb = cpool.tile([P, nblk], BF16)
            nc.sync.dma_start(dloc_sb[:], dloc[:])
            ew_sb = cpool.tile([P, nblk], BF16)
            nc.sync.dma_start(ew_sb[:], ew[:])
            iota_sb = cpool.tile([P, P], BF16)
            nc.sync.dma_start(iota_sb[:], iota[:])
            brep_sb = cpool.tile([P, aggw], F32)
            nc.sync.dma_start(brep_sb[:], brep[:])
            if not second:
                id_sb = cpool.tile([P, P], F32)
                nc.sync.dma_start(id_sb[:], ident[:])
                w2_sb = cpool.tile([nhid, 64], BF16)
                nc.sync.dma_start(w2_sb[:], W2[:])

            msgs_tiles = [[cpool.tile([P, Bg_max, tabw], BF16, tag=f"mt_{c}_{s}", name=f"mt_{c}_{s}")
                           for s in range(NS)] for c in range(n_chunks)]
            qn = 0
            for g in range(ngrp):
                ws = list(range(g * GW, min((g + 1) * GW, nwin)))
                gm = {}
                for c in range(n_chunks):
                    nbc = int(call_nb[g, c])
                    if nbc == 0:
                        continue
                    off = int(call_off[g, c])
                    msgs = msgs_tiles[c][g % NS]
                    gm[c] = (msgs, off)
                    nc.gpsimd.dma_gather(
                        msgs[:, :nbc, :], tab[c * chunk_rows:min((c + 1) * chunk_rows, n_nodes), :],
                        idx_sb[:, off * 8:(off + nbc) * 8],
                        nbc * P, nbc * P, tabw, single_packet=False, queue_num=qn)
                    qn = (qn + 1) % 4
                # per-call fused S build: S[e, blk, d] = w_e * (dloc_e == d)
                sm = {}
                for c in range(n_chunks):
                    nbc = int(call_nb[g, c])
                    if nbc == 0:
                        continue
                    off = int(call_off[g, c])
                    S0 = s0pool.tile([P, Bg_max, P], BF16, tag="s0", name=f"s0_{g}_{c}")
                    nc.vector.tensor_tensor(
                        out=S0[:, :nbc, :],
                        in0=dloc_sb[:, off:off + nbc, None].to_broadcast([P, nbc, P]),
                        in1=iota_sb[:, None, :].to_broadcast([P, nbc, P]),
                        op=mybir.AluOpType.is_equal)
                    S4 = spool.tile([P, Bg_max, P], BF16, tag="s", name=f"s_{g}_{c}")
                    nc.vector.tensor_tensor(
                        out=S4[:, :nbc, :], in0=S0[:, :nbc, :],
                        in1=ew_sb[:, off:off + nbc, None].to_broadcast([P, nbc, P]),
                        op=mybir.AluOpType.mult)
                    sm[c] = S4
                for w in ws:
                    psw = ppool.tile([P, mmw], F32, tag="psw", name=f"psw_{w}")
                    first = True
                    nb_w = int(B[w].sum())
                    if nb_w == 0:
                        nc.vector.memset(psw[:], 0.0)
                    done = 0
                    for c in range(n_chunks):
                        nb = int(B[w, c])
                        if nb == 0:
                            continue
                        msgs, off = gm[c]
                        wo = int(woff[w, c])
                        S4 = sm[c]
                        for b in range(nb):
                            done += 1
                            nc.tensor.matmul(psw[:, :], lhsT=S4[:, wo + b, :], rhs=msgs[:, wo + b, :mmw],
                                             start=first, stop=(done == nb_w))
                            first = False
                    rows = min(P, shard - w * P)
                    if second:
                        o_sb = epool.tile([P, 64], F32, tag="o", name=f"o_{w}")
                        nc.vector.tensor_tensor(out=o_sb[:], in0=psw[:], in1=brep_sb[:], op=mybir.AluOpType.add)
                        nc.sync.dma_start(out[w * P:w * P + rows, :], o_sb[:rows, :])
                    else:
                        hb = epool.tile([P, aggw], F32, tag="hb", name=f"hb_{w}")
                        nc.vector.tensor_tensor(out=hb[:], in0=psw[:], in1=brep_sb[:], op=mybir.AluOpType.add)
                        h = epool.tile([P, aggw], F32, tag="h", name=f"h_{w}")
                        nc.scalar.activation(out=h[:], in_=hb[:], func=mybir.ActivationFunctionType.Relu)
                        pst = p2pool.tile([P, P], F32, tag="pst", name=f"pst_{w}")
                        nc.tensor.transpose(out=pst[:], in_=h[:], identity=id_sb[:])
                        hT = epool.tile([P, P], BF16, tag="hT", name=f"hT_{w}")
                        nc.scalar.activation(out=hT[:], in_=pst[:], func=mybir.ActivationFunctionType.Copy)
                        ps2 = p2pool.tile([P, 64], F32, tag="ps2", name=f"ps2_{w}")
                        nc.tensor.matmul(ps2[:], lhsT=hT[:], rhs=w2_sb[:], start=True, stop=True)
                        o_sb = epool.tile([P, 64], BF16, tag="o", name=f"o_{w}")
                        nc.vector.tensor_copy(out=o_sb[:], in_=ps2[:])
                        nc.sync.dma_start(out[w * P:w * P + rows, :], o_sb[:rows, :])
    nc.compile()
    return nc


# ---------------------------------------------------------------- driver
def gcn_forward(x, edge_index, edge_weight, W1, b1, W2, b2, runner=None):
    """Full forward. runner(nc, in_maps, tag) -> list of per-core output dicts."""
    if runner is None:
        def runner(nc, in_maps, tag):
            res = run_bass_kernel_spmd(nc, in_maps, core_ids=list(range(NCORES)))
            return res.results
    n_nodes, nfeat = x.shape
    nhid = W1.shape[1]
    nclass = W2.shape[1]
    shard = n_nodes // NCORES
    nwin = (shard + P - 1) // P
    chunk_rows = 25000
    n_chunks = (n_nodes + chunk_rows - 1) // chunk_rows
    src = np.asarray(edge_index[0], dtype=np.int64)
    dst = np.asarray(edge_index[1], dtype=np.int64)
    ew = np.asarray(edge_weight, dtype=np.float32)

    per_core, B = bucket_edges(src, dst, ew, n_nodes, n_chunks, chunk_rows, shard, nwin)

    iota = np.tile(np.arange(P, dtype=np.float32), (P, 1)).astype(BF_NP)
    ident = np.eye(P, dtype=np.float32)
    xT = np.ascontiguousarray(np.asarray(x, dtype=np.float32).T.astype(BF_NP))
    W1b = np.asarray(W1, np.float32).astype(BF_NP)

    # phase A
    nc_a = build_phase_a(shard, nfeat, nhid)
    ins_a = [{"xT": np.ascontiguousarray(xT[:, k * shard:(k + 1) * shard]),
              "W1": W1b} for k in range(NCORES)]
    res_a = runner(nc_a, ins_a, "A")
    sup1 = np.ascontiguousarray(
        np.concatenate([r["supT"] for r in res_a], axis=1).T)  # [n_nodes, nhid] bf16

    # phase B
    b1rep = np.tile(np.asarray(b1, np.float32)[None, :], (P, 1))
    W2pad = np.zeros((nhid, 64), np.float32)
    W2pad[:, :nclass] = np.asarray(W2, np.float32)
    W2pad = W2pad.astype(BF_NP)
    nc_b = build_agg(shard, n_chunks, chunk_rows, B, n_nodes, False, nhid)
    ins_b = [{"tab": sup1, "idxs": pc["idx"], "dloc": pc["dloc"], "ew": pc["w"],
              "iota": iota, "ident": ident, "brep": b1rep, "W2": W2pad}
             for pc in per_core]
    res_b = runner(nc_b, ins_b, "B")
    sup2 = np.concatenate([r["out"] for r in res_b], axis=0)  # [n_nodes, 64] bf16
    sup2p = np.zeros((n_nodes, 128), BF_NP)
    sup2p[:, :64] = sup2

    # phase C
    b2rep = np.zeros((P, 64), np.float32)
    b2rep[:, :nclass] = np.asarray(b2, np.float32)[None, :]
    nc_c = build_agg(shard, n_chunks, chunk_rows, B, n_nodes, True, nhid)
    ins_c = [{"tab": sup2p, "idxs": pc["idx"], "dloc": pc["dloc"], "ew": pc["w"],
              "iota": iota, "brep": b2rep}
             for pc in per_core]
    res_c = runner(nc_c, ins_c, "C")
    out = np.concatenate([r["out"] for r in res_c], axis=0)[:, :nclass]
    return np.ascontiguousarray(out.astype(np.float32))


def kernel(x, edge_index, edge_weight, W1, b1, W2, b2):
    """Harness entrypoint: FULL inputs -> FULL output [n_nodes, nclass]."""
    out = gcn_forward(np.asarray(x), np.asarray(edge_index), np.asarray(edge_weight),
                      np.asarray(W1), np.asarray(b1), np.asarray(W2), np.asarray(b2))
    return out.astype(np.float32)


# revision 18
# speedup vs baseline: 2.3939x; 1.1973x over previous
"""2-layer GCN on 8 TRN2 NeuronCores via Bass/Tile.

Strategy (per spec sharding_hint): dst-shard nodes across 8 cores; edges
partitioned by destination; small weight matrices replicated. Three SPMD
launches with host-side shard exchange between them:
  A: supT = (x_shard @ W1)^T in bf16      (host transposes to the table)
  B: h = relu(agg1(sup1)+b1); sup2 = h @ W2   (dst-sharded edges,
     dma_gather per edge from replicated bf16 sup1 table)
  C: out = agg2(sup2) + b2

Aggregation: per 128-edge block, gather rows tab[src] (SWDGE dma_gather,
4 queues, groups of GW=8 dst windows per call), build selection matrix
S[e,d] = w_e * (dstlocal_e == d) in ONE fused DVE tensor_scalar
(out = (iota == dloc) * ew, per-partition scalars), TensorE bf16 matmul
psum[d,f] += S^T @ msgs accumulated over a 128-dst window.
"""
import sys

sys.path.insert(0, "/opt/trn_rl_repo")
import numpy as np
import ml_dtypes
import concourse.bacc as bacc
import concourse.bass as bass
import concourse.mybir as mybir
import concourse.tile as tile
from concourse.bass_utils import run_bass_kernel_spmd
from concourse.library_config import mlp

dt = mybir.dt
F32 = dt.float32
BF16 = dt.bfloat16
BF_NP = ml_dtypes.bfloat16
NCORES = 8
P = 128
GROUP_W = 6  # dst windows per gather-call group


# ---------------------------------------------------------------- host prep
def bucket_edges(src, dst, ew, n_nodes, n_chunks, chunk_rows, shard, nwin):
    """Per-core edge buckets (window = 128 local dsts, chunk = src range).

    Returns per-core arrays (idx wrapped int16, dstloc f32, weight f32,
    all [*, 128]-blocked) plus the uniform block-count table B[w][c].
    Padding uses idx 0 / weight 0 (gathers row 0, killed by S=0), so
    grouped gather calls see a fully valid index stream.
    """
    counts = np.zeros((NCORES, nwin, n_chunks), dtype=np.int64)
    core = dst // shard
    dloc = dst - core * shard
    win = dloc // P
    chunk = src // chunk_rows
    for k in range(NCORES):
        m = core == k
        np.add.at(counts[k], (win[m], chunk[m]), 1)
    B = np.maximum.reduce([np.ceil(counts[k] / P).astype(np.int64) for k in range(NCORES)])
    nblk_tot = int(B.sum())
    nidx_tot = nblk_tot * P

    # per-(group, chunk) call capacities for the valid-count table
    GW = GROUP_W
    ngrp = (nwin + GW - 1) // GW

    per_core = []
    order = np.lexsort((chunk, win, core))
    srt = {"src": src[order], "ew": ew[order], "dloc": dloc[order],
           "core": core[order], "win": win[order], "chunk": chunk[order]}
    for k in range(NCORES):
        idx_arr = np.zeros(nidx_tot, dtype=np.int16)
        dloc_arr = np.zeros(nidx_tot, dtype=np.float32)
        w_arr = np.zeros(nidx_tot, dtype=np.float32)
        sel = srt["core"] == k
        s_src, s_ew, s_dloc = srt["src"][sel], srt["ew"][sel], srt["dloc"][sel]
        s_win, s_chunk = srt["win"][sel], srt["chunk"][sel]
        # bucket start pointers into this core's (win, chunk)-sorted edges
        bstart = np.zeros((nwin, n_chunks + 1), dtype=np.int64)
        csum = 0
        for w in range(nwin):
            for c in range(n_chunks):
                bstart[w, c] = csum
                csum += counts[k, w, c]
            bstart[w, n_chunks] = csum
        # group-major layout: for g (GW windows): for c: for w in g
        cnts = np.zeros((1, ngrp * n_chunks), dtype=np.int32)
        pos = 0
        for g in range(ngrp):
            for c in range(n_chunks):
                for w in range(g * GW, min((g + 1) * GW, nwin)):
                    n = counts[k, w, c]
                    cap = B[w, c] * P
                    e0 = bstart[w, c]
                    e1 = e0 + n
                    idx_arr[pos:pos + n] = (s_src[e0:e1] - c * chunk_rows).astype(np.int16)
                    dloc_arr[pos:pos + n] = (s_dloc[e0:e1] - w * P).astype(np.float32)
                    w_arr[pos:pos + n] = s_ew[e0:e1]
                    # padding: idx 0, weight 0 kills the gathered row
                    cnts[0, g * n_chunks + c] += n
                    pos += cap
        idx_wrapped = np.tile(idx_arr.reshape(-1, 16).T, (8, 1)).copy()  # [128, n/16]
        per_core.append({
            "idx": idx_wrapped,
            "dloc": dloc_arr.reshape(-1, P).T.astype(BF_NP),  # [128, nblk]
            "w": w_arr.reshape(-1, P).T.astype(BF_NP),        # [128, nblk]
            "cnt": cnts,
        })
    return per_core, B


# ---------------------------------------------------------------- phase A
def build_phase_a(shard, nfeat, nhid):
    """supT shard = (x_shard @ W1)^T in bf16; host transposes to table."""
    nc = bacc.Bacc("TRN2")
    xT = nc.declare_dram_parameter("xT", [nfeat, shard], BF16, isOutput=False)
    W1 = nc.declare_dram_parameter("W1", [nfeat, nhid], BF16, isOutput=False)
    supT = nc.declare_dram_parameter("supT", [nhid, shard], BF16, isOutput=True)
    kt = nfeat // P
    NT = 512
    ntiles = (shard + NT - 1) // NT
    with tile.TileContext(nc) as tc:
        with (
            tc.tile_pool(name="const", bufs=1) as cpool,
            tc.tile_pool(name="work", bufs=3) as wpool,
            tc.tile_pool(name="psum", bufs=2, space="PSUM") as ppool,
        ):
            w1_sb = [cpool.tile([P, nhid], BF16, tag=f"w1_{k}", name=f"w1_{k}") for k in range(kt)]
            for k in range(kt):
                nc.sync.dma_start(w1_sb[k][:], W1[k * P:(k + 1) * P, :])
            for t in range(ntiles):
                n0 = t * NT
                n = min(NT, shard - n0)
                xt_sb = [wpool.tile([P, NT], BF16, tag=f"xt_{k}", name=f"xt_{k}_{t}") for k in range(kt)]
                for k in range(kt):
                    nc.sync.dma_start(xt_sb[k][:, :n], xT[k * P:(k + 1) * P, n0:n0 + n])
                psT = ppool.tile([P, NT], F32, tag="ps")
                for k in range(kt):
                    nc.tensor.matmul(psT[:, :n], lhsT=w1_sb[k][:],
                                     rhs=xt_sb[k][:, :n],
                                     start=(k == 0), stop=(k == kt - 1))
                supT_sb = wpool.tile([P, NT], BF16, tag="supT")
                nc.vector.tensor_copy(out=supT_sb[:, :n], in_=psT[:, :n])
                nc.sync.dma_start(supT[:, n0:n0 + n], supT_sb[:, :n])
    nc.compile()
    return nc


# ---------------------------------------------------------------- phases B/C
def build_agg(shard, n_chunks, chunk_rows, B, n_nodes, second, nhid):
    """Aggregation kernel over a bf16 [n_nodes, 128] gather table.

    second=False (phase B): epilogue h=relu(agg+b1); sup2 = h @ W2pad
      -> out [shard, 64] bf16.
    second=True (phase C): agg over first 64 cols; out = agg + b2
      -> out [shard, 64] f32.
    """
    nwin = B.shape[0]
    nblk = int(B.sum())
    nidx = nblk * P
    nc = bacc.Bacc("TRN2", num_swdge_queues=4)
    tabw = 128
    mmw = 64 if second else 128
    aggw = 64 if second else 128
    tab = nc.declare_dram_parameter("tab", [n_nodes, tabw], BF16, isOutput=False)
    idxs = nc.declare_dram_parameter("idxs", [P, nidx // 16], dt.int16, isOutput=False)
    dloc = nc.declare_dram_parameter("dloc", [P, nblk], BF16, isOutput=False)
    ew = nc.declare_dram_parameter("ew", [P, nblk], BF16, isOutput=False)
    brep = nc.declare_dram_parameter("brep", [P, aggw], F32, isOutput=False)
    if not second:
        ident = nc.declare_dram_parameter("ident", [P, P], F32, isOutput=False)
        W2 = nc.declare_dram_parameter("W2", [nhid, 64], BF16, isOutput=False)
    out = nc.declare_dram_parameter("out", [shard, 64], F32 if second else BF16,
                                    isOutput=True)

    # grouped-call layout: for g (GW windows): for c: for w in g: B[w,c] blocks
    GW = GROUP_W
    ngrp = (nwin + GW - 1) // GW
    call_off = np.zeros((ngrp, n_chunks), dtype=np.int64)   # call start block
    call_nb = np.zeros((ngrp, n_chunks), dtype=np.int64)    # blocks per call
    woff = np.zeros((nwin, n_chunks), dtype=np.int64)       # window offset within call
    acc = 0
    for g in range(ngrp):
        ws = range(g * GW, min((g + 1) * GW, nwin))
        for c in range(n_chunks):
            call_off[g, c] = acc
            o = 0
            for w in ws:
                woff[w, c] = o
                o += B[w, c]
            call_nb[g, c] = o
            acc += o
    Bg_max = int(call_nb.max())

    iota_exp = nc.declare_dram_parameter("iota_exp", [P, P * Bg_max], BF16, isOutput=False)
    cnt = nc.declare_dram_parameter("cnt", [1, ngrp * n_chunks], dt.int32, isOutput=False)

    NS = 2

    with tile.TileContext(nc) as tc:
        nc.gpsimd.load_library(mlp)
        with (
            tc.tile_pool(name="const", bufs=1) as cpool,
            tc.tile_pool(name="s0", bufs=2) as s0pool,
            tc.tile_pool(name="s", bufs=2 * n_chunks) as spool,
            tc.tile_pool(name="epi", bufs=3) as epool,
            tc.tile_pool(name="psum", bufs=3, space="PSUM") as ppool,
            tc.tile_pool(name="psum2", bufs=2, space="PSUM") as p2pool,
        ):
            idx_sb = cpool.tile([P, nidx // 16], dt.int16)
            nc.sync.dma_start(idx_sb[:], idxs[:])
            dloc_sb = cpool.tile([P, nblk], BF16)
            nc.sync.dma_start(dloc_sb[:], dloc[:])
            ew_sb = cpool.tile([P, nblk], BF16)
            nc.sync.dma_start(ew_sb[:], ew[:])
            iota_sb = cpool.tile([P, P, Bg_max], BF16)
            nc.sync.dma_start(iota_sb[:], iota_exp[:])
            brep_sb = cpool.tile([P, aggw], F32)
            nc.sync.dma_start(brep_sb[:], brep[:])
            cnt_sb = cpool.tile([1, ngrp * n_chunks], dt.int32)
            nc.sync.dma_start(cnt_sb[:], cnt[:])
            if not second:
                id_sb = cpool.tile([P, P], F32)
                nc.sync.dma_start(id_sb[:], ident[:])
                w2_sb = cpool.tile([nhid, 64], BF16)
                nc.sync.dma_start(w2_sb[:], W2[:])

            msgs_tiles = [[cpool.tile([P, Bg_max, tabw], BF16, tag=f"mt_{c}_{s}", name=f"mt_{c}_{s}")
                           for s in range(NS)] for c in range(n_chunks)]
            qn = 0
            for g in range(ngrp):
                ws = list(range(g * GW, min((g + 1) * GW, nwin)))
                gm = {}
                for c in range(n_chunks):
                    nbc = int(call_nb[g, c])
                    if nbc == 0:
                        continue
                    off = int(call_off[g, c])
                    msgs = msgs_tiles[c][g % NS]
                    gm[c] = (msgs, off)
                    nc.gpsimd.dma_gather(
                        msgs[:, :nbc, :], tab[c * chunk_rows:min((c + 1) * chunk_rows, n_nodes), :],
                        idx_sb[:, off * 8:(off + nbc) * 8],
                        nbc * P, nbc * P, tabw, single_packet=False, queue_num=qn)
                    qn = (qn + 1) % 4
                # per-call fused S build (d-major, packed last dim for DVE 2x):
                # S[e, d, blk] = w_e * (dloc_e == d)
                sm = {}
                for c in range(n_chunks):
                    nbc = int(call_nb[g, c])
                    if nbc == 0:
                        continue
                    off = int(call_off[g, c])
                    S0 = s0pool.tile([P, P, Bg_max], BF16, tag="s0", name=f"s0_{g}_{c}")
                    nc.vector.tensor_tensor(
                        out=S0[:, :, :nbc],
                        in0=dloc_sb[:, None, off:off + nbc].to_broadcast([P, P, nbc]),
                        in1=iota_sb[:, :, :nbc],
                        op=mybir.AluOpType.is_equal)
                    S4 = spool.tile([P, P, Bg_max], BF16, tag="s", name=f"s_{g}_{c}")
                    nc.vector.tensor_tensor(
                        out=S4[:, :, :nbc], in0=S0[:, :, :nbc],
                        in1=ew_sb[:, None, off:off + nbc].to_broadcast([P, P, nbc]),
                        op=mybir.AluOpType.mult)
                    sm[c] = S4
                for w in ws:
                    psw = ppool.tile([P, mmw], F32, tag="psw", name=f"psw_{w}")
                    first = True
                    nb_w = int(B[w].sum())
                    if nb_w == 0:
                        nc.vector.memset(psw[:], 0.0)
                    done = 0
                    for c in range(n_chunks):
                        nb = int(B[w, c])
                        if nb == 0:
                            continue
                        msgs, off = gm[c]
                        wo = int(woff[w, c])
                        S4 = sm[c]
                        for b in range(nb):
                            done += 1
                            nc.tensor.matmul(psw[:, :], lhsT=S4[:, :, wo + b], rhs=msgs[:, wo + b, :mmw],
                                             start=first, stop=(done == nb_w))
                            first = False
                    rows = min(P, shard - w * P)
                    if second:
                        o_sb = epool.tile([P, 64], F32, tag="o", name=f"o_{w}")
                        nc.vector.tensor_tensor(out=o_sb[:], in0=psw[:], in1=brep_sb[:], op=mybir.AluOpType.add)
                        nc.sync.dma_start(out[w * P:w * P + rows, :], o_sb[:rows, :])
                    else:
                        hb = epool.tile([P, aggw], F32, tag="hb", name=f"hb_{w}")
                        nc.vector.tensor_tensor(out=hb[:], in0=psw[:], in1=brep_sb[:], op=mybir.AluOpType.add)
                        h = epool.tile([P, aggw], F32, tag="h", name=f"h_{w}")
                        nc.scalar.activation(out=h[:], in_=hb[:], func=mybir.ActivationFunctionType.Relu)
                        pst = p2pool.tile([P, P], F32, tag="pst", name=f"pst_{w}")
                        nc.tensor.transpose(out=pst[:], in_=h[:], identity=id_sb[:])
                        hT = epool.tile([P, P], BF16, tag="hT", name=f"hT_{w}")
                        nc.scalar.activation(out=hT[:], in_=pst[:], func=mybir.ActivationFunctionType.Copy)
                        ps2 = p2pool.tile([P, 64], F32, tag="ps2", name=f"ps2_{w}")
                        nc.tensor.matmul(ps2[:], lhsT=hT[:], rhs=w2_sb[:], start=True, stop=True)
                        o_sb = epool.tile([P, 64], BF16, tag="o", name=f"o_{w}")
                        nc.vector.tensor_copy(out=o_sb[:], in_=ps2[:])
                        nc.sync.dma_start(out[w * P:w * P + rows, :], o_sb[:rows, :])
    nc.compile()
    return nc, Bg_max


# ---------------------------------------------------------------- driver
def gcn_forward(x, edge_index, edge_weight, W1, b1, W2, b2, runner=None):
    """Full forward. runner(nc, in_maps, tag) -> list of per-core output dicts."""
    if runner is None:
        def runner(nc, in_maps, tag):
            res = run_bass_kernel_spmd(nc, in_maps, core_ids=list(range(NCORES)))
            return res.results
    n_nodes, nfeat = x.shape
    nhid = W1.shape[1]
    nclass = W2.shape[1]
    shard = n_nodes // NCORES
    nwin = (shard + P - 1) // P
    chunk_rows = 25000
    n_chunks = (n_nodes + chunk_rows - 1) // chunk_rows
    src = np.asarray(edge_index[0], dtype=np.int64)
    dst = np.asarray(edge_index[1], dtype=np.int64)
    ew = np.asarray(edge_weight, dtype=np.float32)

    per_core, B = bucket_edges(src, dst, ew, n_nodes, n_chunks, chunk_rows, shard, nwin)

    ident = np.eye(P, dtype=np.float32)
    xT = np.ascontiguousarray(np.asarray(x, dtype=np.float32).T.astype(BF_NP))
    W1b = np.asarray(W1, np.float32).astype(BF_NP)

    # phase A
    nc_a = build_phase_a(shard, nfeat, nhid)
    ins_a = [{"xT": np.ascontiguousarray(xT[:, k * shard:(k + 1) * shard]),
              "W1": W1b} for k in range(NCORES)]
    res_a = runner(nc_a, ins_a, "A")
    sup1 = np.ascontiguousarray(
        np.concatenate([r["supT"] for r in res_a], axis=1).T)  # [n_nodes, nhid] bf16

    # phase B
    b1rep = np.tile(np.asarray(b1, np.float32)[None, :], (P, 1))
    W2pad = np.zeros((nhid, 64), np.float32)
    W2pad[:, :nclass] = np.asarray(W2, np.float32)
    W2pad = W2pad.astype(BF_NP)
    nc_b, bg_max = build_agg(shard, n_chunks, chunk_rows, B, n_nodes, False, nhid)
    iota_exp = np.repeat(np.arange(P, dtype=np.float32), bg_max)[None, :].repeat(P, 0).astype(BF_NP)
    ins_b = [{"tab": sup1, "idxs": pc["idx"], "dloc": pc["dloc"], "ew": pc["w"],
              "iota_exp": iota_exp, "cnt": pc["cnt"], "ident": ident,
              "brep": b1rep, "W2": W2pad}
             for pc in per_core]
    res_b = runner(nc_b, ins_b, "B")
    sup2 = np.concatenate([r["out"] for r in res_b], axis=0)  # [n_nodes, 64] bf16
    sup2p = np.zeros((n_nodes, 128), BF_NP)
    sup2p[:, :64] = sup2

    # phase C
    b2rep = np.zeros((P, 64), np.float32)
    b2rep[:, :nclass] = np.asarray(b2, np.float32)[None, :]
    nc_c, _ = build_agg(shard, n_chunks, chunk_rows, B, n_nodes, True, nhid)
    ins_c = [{"tab": sup2p, "idxs": pc["idx"], "dloc": pc["dloc"], "ew": pc["w"],
              "iota_exp": iota_exp, "cnt": pc["cnt"], "brep": b2rep}
             for pc in per_core]
    res_c = runner(nc_c, ins_c, "C")
    out = np.concatenate([r["out"] for r in res_c], axis=0)[:, :nclass]
    return np.ascontiguousarray(out.astype(np.float32))


def kernel(x, edge_index, edge_weight, W1, b1, W2, b2):
    """Harness entrypoint: FULL inputs -> FULL output [n_nodes, nclass]."""
    out = gcn_forward(np.asarray(x), np.asarray(edge_index), np.asarray(edge_weight),
                      np.asarray(W1), np.asarray(b1), np.asarray(W2), np.asarray(b2))
    return out.astype(np.float32)
